# revision 65
# baseline (speedup 1.0000x reference)
"""Multi-head cosine self-attention on 8 Trainium2 NeuronCores (Bass/Tile).

Problem: y = MHA(x) with L2-normalized q/k (cosine attention) and per-head
scaling sim / n**sigmoid(m);  x: [4, 2048, 1024], 16 heads of dim 64.

There is no softmax, so attention is LINEAR and can be reassociated:
    out_h = (q̂_h k̂_hᵀ / s_h) v_h = q̃_h (k_hᵀ ṽ_h)
with  ṽ_h = v_h / ||k_j||  (k-norms folded into v rows) and
      q̃_h = q_h / (s_h ||q_i||)  (q-norms and head scale folded into q).
The O(n²) sim/attn matrices disappear; per head only a 64×64 kv product
remains.  Folding further, per-head A_h = kv_hᵀ Wo_h stacks into M [512, F]
so the entire attention + output projection collapses to  y_part = q̃ M.

Sharding: core c handles batch c//2 and head-group c%2 (8 heads = 512 of
the 1024 q/k/v features).  Host sums the two partials per batch, adds bo.

Per-core pipeline (n=2048; matmul operands bf16, PSUM f32).  HW profile
shows the PE streams N=512 matmuls back-to-back at ~216 ns once warm, so
the design minimizes everything around the matmul stream:
  - the startup-critical k-burst inputs (wk + x chunk 0) are spread over
    ALL THREE DMA queues in 256KB slices (per-queue bandwidth saturates
    below the core's share); wv/wq queue strictly behind them, x
    prefetches ride behind wq, wo is deferred to chunk 1.  x and out use
    chunk/tile-major DRAM layouts so every DMA is contiguous.
  - within each projection tile pair the contraction loop runs k8-outer
    so matmul k8 consumes exactly the weight slice the DMA just
    delivered (chunk 0 tracks the arrival curve).
  - ONE unified PSUM pool for all phases (tags: big [P,2,512]x3 = 6
    banks, kv 1 bank, qn 1 bank) so there is no pool-transition barrier
    between the projection phase and the M/output phases.
  - q-norm scale = Abs_reciprocal_sqrt in ONE scalar-engine op (bf16
    out) — the old Sqrt + DVE-reciprocal chain cost 3.3us on the
    8-partition [8,512] layout and stalled the PE FIFO at the phase
    transition.  ind8/qnr are bf16 so the broadcast matmuls run at
    normal bf16 speed instead of fp32-HIGH.
  - transition order: kv block-diagonal evicts first (vector), last
    q-norm reduce, then M = blkT Wo matmuls cover the rsqrt latency,
    then the last q-scale apply, then phase 4; the last chunk's qT
    copies are deferred past the rsqrt in the scalar FIFO.
  - phase-4 output tiles evict per 512-feature half (DVE fc0 / ACT fc1)
    into one contiguous staging tile, then ONE 256KB DMA per n-tile, all
    on the sync queue (gpsimd then has no outstanding transfers at
    program end, collapsing its ~3us epilogue DRAIN); tile-major DRAM
    layout so the host reshape is a view, last tile in two halves.
"""

import os
import sys

for _p in ("/opt/trn_rl_repo",):
    if os.path.isdir(_p) and _p not in sys.path:
        sys.path.insert(0, _p)

from contextlib import ExitStack

import ml_dtypes
import numpy as np

import concourse.bacc as bacc
import concourse.mybir as mybir
import concourse.tile as tile
from concourse import bass_utils

P = 128
F = 1024  # model dim
H = 16  # total heads
HD = 64  # head dim
G = 2  # head groups (tensor-parallel factor)
FG = F // G  # 512 features per core
PAIRS = FG // P  # 4 head-pairs per core
KT = F // P  # 8 contraction tiles for the projections
NCORES = 8
F32 = mybir.dt.float32
FR = mybir.dt.float32r
BF = mybir.dt.bfloat16
AF = mybir.ActivationFunctionType
MUL = mybir.AluOpType.mult
ADD = mybir.AluOpType.add


def _mm(nc, out, lhsT, rhs, **kw):
    nc.tensor.matmul(out, lhsT, rhs, **kw)


def build_core_program(nc, n=2048, reps=1, zero_bias=True):
    NC = n // 512  # i-chunks
    NT = n // P  # n-tiles
    NTC = 512 // P  # n-tiles per i-chunk

    # chunk-major x layout: each 512-row chunk is a contiguous 8KB/partition
    # block, so chunk DMAs stream at full line efficiency
    xt = nc.dram_tensor("xt", [P, n // 512, KT, 512], BF,
                        kind="ExternalInput").ap()
    wq = nc.dram_tensor("wq", [P, KT, PAIRS, P], BF, kind="ExternalInput").ap()
    wk = nc.dram_tensor("wk", [P, KT, FG], BF, kind="ExternalInput").ap()
    wv = nc.dram_tensor("wv", [P, KT, FG], BF, kind="ExternalInput").ap()
    wo = nc.dram_tensor("wo", [P, PAIRS, F], BF, kind="ExternalInput").ap()
    bqd = nc.dram_tensor("bq", [P, PAIRS], F32, kind="ExternalInput").ap()
    bkd = nc.dram_tensor("bk", [1, FG], BF, kind="ExternalInput").ap()
    bvd = nc.dram_tensor("bv", [1, FG], BF, kind="ExternalInput").ap()
    # cmblk[:, p, l]: s²_{2p+a} on the partition block of head a=l-2p (else 0)
    cmblk = nc.dram_tensor("cmblk", [P, PAIRS, 8], BF, kind="ExternalInput").ap()
    # ind8[l, p, m] = 1 iff l == 2p + m//64  (partition-broadcast selector)
    ind8 = nc.dram_tensor("ind8", [8, PAIRS, P], BF, kind="ExternalInput").ap()
    cones = nc.dram_tensor("cones", [1, P], BF, kind="ExternalInput").ap()
    # tile-major out layout: each [128, 1024] n-tile is one contiguous
    # 256KB block (host reshape is a no-op view)
    out = nc.dram_tensor("out", [n // P, P, F], BF,
                         kind="ExternalOutput").ap()

    with tile.TileContext(nc) as tc, ExitStack() as ctx:
        const = ctx.enter_context(tc.tile_pool(name="const", bufs=1))
        persist = ctx.enter_context(tc.tile_pool(name="persist", bufs=1))
        work = ctx.enter_context(tc.tile_pool(name="work", bufs=1))

        cm_sb = const.tile([P, PAIRS, 8], BF)
        ind_sb = const.tile([8, PAIRS, P], BF)
        zcol = const.tile([P, 1], F32)
        nc.any.memset(zcol[:], 0.0)
        if not zero_bias:
            ones_row = const.tile([1, P], BF)
            nc.sync.dma_start(ones_row[:], cones)
            bq_sb = const.tile([P, PAIRS], F32)
            nc.sync.dma_start(bq_sb[:], bqd)
            bk_sb = const.tile([1, FG], BF)
            nc.sync.dma_start(bk_sb[:], bkd)
            bv_sb = const.tile([1, FG], BF)
            nc.sync.dma_start(bv_sb[:], bvd)

        # --- weights + first x chunk: per-k8 128KB slices so the first
        # matmuls can start as soon as slice 0 lands.  Queues: scalar =
        # wk then wq then cm/ind; sync = xc0 then later x prefetches +
        # half the out tiles; gpsimd = wv only; vector = wo (deferred to
        # chunk 1) + the other half of out tiles. ------------------------
        wk_sb = persist.tile([P, KT, FG], BF)
        wv_sb = persist.tile([P, KT, FG], BF)
        wq_sb = persist.tile([P, KT, PAIRS, P], BF)
        wo_sb = persist.tile([P, PAIRS, F], BF)
        xc0 = work.tile([P, KT, 512], BF, tag="x", bufs=3)
        # k-burst inputs (wk + xc0) spread over ALL THREE queues first —
        # per-queue bandwidth saturates well below the core's share, so
        # three concurrent streams feed the first matmuls fastest.  wv/wq
        # queue strictly behind them, cm/ind on the then-idle sync queue.
        nc.scalar.dma_start(wk_sb[:, 0:2], wk[:, 0:2])
        nc.sync.dma_start(xc0[:, 0:2], xt[:, 0, 0:2])
        nc.gpsimd.dma_start(wk_sb[:, 2:4], wk[:, 2:4])
        nc.scalar.dma_start(xc0[:, 2:4], xt[:, 0, 2:4])
        nc.sync.dma_start(wk_sb[:, 4:6], wk[:, 4:6])
        nc.gpsimd.dma_start(xc0[:, 4:6], xt[:, 0, 4:6])
        nc.scalar.dma_start(wk_sb[:, 6:8], wk[:, 6:8])
        nc.sync.dma_start(xc0[:, 6:8], xt[:, 0, 6:8])
        for k8 in range(0, KT, 2):
            nc.gpsimd.dma_start(wv_sb[:, k8:k8 + 2], wv[:, k8:k8 + 2])
        for k8 in range(0, KT, 2):
            nc.scalar.dma_start(wq_sb[:, k8:k8 + 2], wq[:, k8:k8 + 2])
        nc.sync.dma_start(cm_sb[:], cmblk)
        nc.sync.dma_start(ind_sb[:], ind8)

        # --- persistent activations -------------------------------------
        qT = persist.tile([P, PAIRS, n], BF)  # q̃^T (scaled in place)
        sq = persist.tile([P, PAIRS, n], BF)  # (q+bq)^2
        ksb = persist.tile([P, NT, FG], BF)  # k natural
        vsb = persist.tile([P, NT, FG], BF)  # ṽ natural (k-norms folded)
        knf = persist.tile([P, NT, 8], F32)  # 1/||k_j|| per local head
        blk = persist.tile([P, PAIRS, P], BF)  # block-diag kvT per pair
        nc.any.memset(blk[:], 0.0)
        msb = persist.tile([P, PAIRS, F], BF)  # M = kvT-blk^T @ Wo rows

        with tc.tile_pool(name="ps", bufs=1, space="PSUM") as ps:

            def qn_apply_block(ic, qnr):
                # broadcast across partitions + scale qT in place
                icsl = slice(ic * 512, (ic + 1) * 512)
                for h in range(PAIRS // 2):
                    bps = ps.tile([P, 2, 512], F32, tag="big", bufs=3)
                    for u in range(2):
                        pr = 2 * h + u
                        _mm(nc, bps[:, u, :], ind_sb[:, pr, :], qnr[:],
                            start=True, stop=True)
                    psl = slice(2 * h, 2 * h + 2)
                    nc.vector.tensor_tensor(qT[:, psl, icsl],
                                            qT[:, psl, icsl], bps, MUL)

            def qn_norm_block(ic):
                icsl = slice(ic * 512, (ic + 1) * 512)
                qnp = ps.tile([8, 512], F32, tag="qn", bufs=1)
                for pr in range(PAIRS):
                    _mm(nc, qnp, cm_sb[:, pr, :], sq[:, pr, icsl],
                        start=(pr == 0), stop=(pr == PAIRS - 1))
                qnr = work.tile([8, 512], BF, tag="qnr", bufs=2)
                nc.scalar.activation(qnr[:], qnp, AF.Abs_reciprocal_sqrt,
                                     bias=zcol[:8])
                return qnr

            for _rep in range(reps):  # reps>1: device-time measurement only
                # ========= phase 1: projections + norms, per i-chunk =====
                # The q-norm/scale block for chunk ic is deferred one chunk
                # so its PE matmuls never wait on the ACT/DVE chain.
                xc_next = xc0 if _rep == 0 else None
                for ic in range(NC):
                    icsl = slice(ic * 512, (ic + 1) * 512)
                    ntsl = slice(ic * NTC, (ic + 1) * NTC)
                    # x prefetches ride the scalar queue BEHIND wk/wq/cm:
                    # per-queue FIFO transfer order keeps them from stealing
                    # round-robin bandwidth from the startup-critical
                    # streams (sync=xc0, scalar=wk, gpsimd=wv).
                    if xc_next is None:
                        xc = work.tile([P, KT, 512], BF, tag="x", bufs=3)
                        nc.scalar.dma_start(xc[:], xt[:, ic])
                    else:
                        xc = xc_next
                    if ic + 1 < NC:  # prefetch next chunk
                        xc_next = work.tile([P, KT, 512], BF, tag="x", bufs=3)
                        nc.scalar.dma_start(xc_next[:], xt[:, ic + 1])
                    else:
                        xc_next = None
                    if ic == 1 and _rep == 0:
                        nc.gpsimd.dma_start(wo_sb[:], wo)

                    # ---- k tiles (natural layout), 2 per PSUM tile,
                    # k8-outer so chunk 0 tracks the wk DMA slices --------
                    for h in range(NTC // 2):
                        kp = ps.tile([P, 2, 512], F32, tag="big", bufs=3)
                        for k8 in range(KT):
                            for u in range(2):
                                jt = 2 * h + u
                                jsl = slice(jt * P, (jt + 1) * P)
                                _mm(nc, kp[:, u, :], xc[:, k8, jsl],
                                    wk_sb[:, k8, :], start=(k8 == 0),
                                    stop=(zero_bias and k8 == KT - 1))
                        if not zero_bias:
                            for u in range(2):
                                _mm(nc, kp[:, u, :], ones_row, bk_sb,
                                    start=False, stop=True)
                        nc.vector.tensor_copy(
                            ksb[:, ic * NTC + 2 * h:ic * NTC + 2 * h + 2, :], kp)
                    # deferred q-norm reduce of the previous chunk
                    if ic >= 1:
                        qnr_prev = qn_norm_block(ic - 1)
                    # ---- k-norm chain, whole chunk ----------------------
                    ksq = work.tile([P, NTC, FG], BF, tag="ksq", bufs=1)
                    nc.gpsimd.tensor_mul(ksq[:], ksb[:, ntsl, :],
                                         ksb[:, ntsl, :])
                    kn2 = work.tile([P, NTC * 8], F32, tag="kn2", bufs=1)
                    nc.vector.tensor_reduce(
                        kn2[:], ksq.rearrange("p t (h d) -> p t h d", h=8),
                        mybir.AxisListType.X, ADD)
                    nc.scalar.activation(
                        knf[:, ntsl, :].rearrange("p t h -> p (t h)"), kn2[:],
                        AF.Abs_reciprocal_sqrt, bias=zcol[:])
                    # ---- v tiles ----------------------------------------
                    for h in range(NTC // 2):
                        vp = ps.tile([P, 2, 512], F32, tag="big", bufs=3)
                        for k8 in range(KT):
                            for u in range(2):
                                jt = 2 * h + u
                                jsl = slice(jt * P, (jt + 1) * P)
                                _mm(nc, vp[:, u, :], xc[:, k8, jsl],
                                    wv_sb[:, k8, :], start=(k8 == 0),
                                    stop=(zero_bias and k8 == KT - 1))
                        if not zero_bias:
                            for u in range(2):
                                _mm(nc, vp[:, u, :], ones_row, bv_sb,
                                    start=False, stop=True)
                        nc.scalar.copy(
                            vsb[:, ic * NTC + 2 * h:ic * NTC + 2 * h + 2, :], vp)
                    # fold 1/||k_j|| into v rows (GpSimd, SBUF-only)
                    nc.gpsimd.tensor_tensor(
                        vsb[:, ntsl, :].rearrange("p t (h d) -> p t h d", h=8),
                        vsb[:, ntsl, :].rearrange("p t (h d) -> p t h d", h=8),
                        knf[:, ntsl, :].unsqueeze(-1).broadcast_to(
                            [P, NTC, 8, HD]), MUL)

                    # ---- qT pairs (transposed layout), 2 per PSUM tile --
                    # last chunk: Squares first, qT copies deferred past the
                    # transition rsqrt so it isn't stuck behind them in the
                    # scalar FIFO (the qp PSUM bufs are WAR-protected)
                    qp_last = []
                    for h in range(PAIRS // 2):
                        qp = ps.tile([P, 2, 512], F32, tag="big", bufs=3)
                        for k8 in range(KT):
                            for u in range(2):
                                pr = 2 * h + u
                                _mm(nc, qp[:, u, :], wq_sb[:, k8, pr, :],
                                    xc[:, k8, :], start=(k8 == 0),
                                    stop=(k8 == KT - 1))
                        psl = slice(2 * h, 2 * h + 2)
                        if zero_bias and ic == NC - 1:
                            nc.scalar.activation(sq[:, psl, icsl], qp,
                                                 AF.Square, bias=zcol[:])
                            qp_last.append((qp, psl))
                        elif zero_bias:
                            # sq = qT*qT on the lightly-loaded GpSimd
                            # (SBUF-only) — halves the scalar engine's
                            # per-chunk load; the one-chunk-deferred qn
                            # block gives the longer chain plenty of slack
                            nc.scalar.copy(qT[:, psl, icsl], qp)
                            nc.gpsimd.tensor_mul(sq[:, psl, icsl],
                                                 qT[:, psl, icsl],
                                                 qT[:, psl, icsl])
                        else:
                            for u in range(2):
                                pr = 2 * h + u
                                nc.scalar.activation(
                                    qT[:, pr, icsl], qp[:, u, :], AF.Identity,
                                    bias=bq_sb[:, pr:pr + 1])
                                nc.scalar.activation(
                                    sq[:, pr, icsl], qp[:, u, :], AF.Square,
                                    bias=bq_sb[:, pr:pr + 1])

                    # ---- apply the previous chunk's q scaling -----------
                    if ic >= 1:
                        qn_apply_block(ic - 1, qnr_prev)

                    # ---- incremental kvT accumulation for this chunk ----
                    # kvT = ṽ^T k in one persistent PSUM bank: one full-pair
                    # matmul per (pair, n-tile) computes the two good
                    # diagonal 64×64 blocks (plus off-diagonal garbage that
                    # is never evicted), accumulated across all chunks.
                    if ic == 0:
                        kvp = ps.tile([P, PAIRS, P], F32, tag="kv", bufs=1)
                    for pr in range(PAIRS):
                        c0 = pr * P
                        for nt in range(ic * NTC, (ic + 1) * NTC):
                            # single start=True: the bank-wide has_written
                            # clear must precede ALL pairs' first writes
                            _mm(nc, kvp[:, pr, :], vsb[:, nt, c0:c0 + P],
                                ksb[:, nt, c0:c0 + P],
                                start=(nt == 0 and pr == 0),
                                stop=(nt == NT - 1),
                                skip_group_check=True)

                # ========= transition: last q-norm + kv evict + M ========
                # PE order: last chunk's qn reduce matmuls, then its apply
                # matmuls — DVE+ACT evict the kv diagonal blocks underneath
                # — then M = blkT Wo with no pool barrier.
                # both kv evicts on the (now light) scalar queue: vector's
                # chunk-3 backlog (k evicts + apply-TT + qT copy) was
                # delaying them ~1.3us, gating the M matmuls
                nc.scalar.copy(blk[0:HD, :, 0:HD], kvp[0:HD, :, 0:HD])
                nc.scalar.copy(blk[HD:P, :, HD:P], kvp[HD:P, :, HD:P])
                qnr_last = qn_norm_block(NC - 1)
                # both deferred qT copies on VECTOR: on scalar they queue
                # behind the rsqrt's wait and stall the M matmuls via the
                # PSUM-ring WAR (mp tiles alias the qp tiles)
                lsl = slice((NC - 1) * 512, NC * 512)
                for qp, psl in qp_last:
                    nc.vector.tensor_copy(qT[:, psl, lsl], qp)

                # ========= phase 3: M = blk^T @ Wo rows ==================
                # (emitted before the last qn apply: the PE FIFO then has
                # ~2us of M matmuls to chew on while the Rsqrt lands)
                for pr in range(PAIRS):
                    mp = ps.tile([P, 2, 512], F32, tag="big", bufs=3)
                    for fc in range(2):
                        fsl = slice(fc * 512, (fc + 1) * 512)
                        _mm(nc, mp[:, fc, :], blk[:, pr, :],
                            wo_sb[:, pr, fsl], start=True, stop=True)
                    # evict per 512-half, rotating over three engines, so
                    # phase 4's first accumulation group is unblocked as
                    # early as possible
                    for fc in range(2):
                        fsl = slice(fc * 512, (fc + 1) * 512)
                        if (2 * pr + fc) % 2 == 0:
                            nc.vector.tensor_copy(msb[:, pr, fsl], mp[:, fc, :])
                        else:
                            nc.scalar.copy(msb[:, pr, fsl], mp[:, fc, :])

                qn_apply_block(NC - 1, qnr_last)

                # ========= phase 4: y = q̃^T.T @ M, per n-tile ============
                for nt in range(NT):
                    ntsl = slice(nt * P, (nt + 1) * P)
                    op = ps.tile([P, 2, 512], F32, tag="big", bufs=3)
                    # one contiguous staging tile per n-tile: vector evicts
                    # the fc0 half, scalar the fc1 half, then a single DMA
                    # per tile (alternating queues; quarters on the last
                    # tile so the final drain is short)
                    ost = work.tile([P, 2, 512], BF, tag="ost", bufs=4)
                    for fc in range(2):
                        fsl = slice(fc * 512, (fc + 1) * 512)
                        for pr in range(PAIRS):
                            _mm(nc, op[:, fc, :], qT[:, pr, ntsl],
                                msb[:, pr, fsl],
                                start=(pr == 0), stop=(pr == PAIRS - 1))
                        if fc == 0:
                            nc.vector.tensor_copy(ost[:, 0, :], op[:, fc, :])
                        else:
                            nc.scalar.copy(ost[:, 1, :], op[:, fc, :])
                    # all out-DMAs on the sync queue: gpsimd then has no
                    # outstanding transfers at program end, so its epilogue
                    # DRAIN (measured ~3us) collapses
                    ostf = ost.rearrange("p u f -> p (u f)")
                    if nt == NT - 1:
                        nc.sync.dma_start(out[nt, :, 0:512], ost[:, 0, :])
                        nc.sync.dma_start(out[nt, :, 512:1024], ost[:, 1, :])
                    else:
                        nc.sync.dma_start(out[nt], ostf)
    return nc


_CACHE = {}


def get_nc(n=2048, zero_bias=True):
    key = (n, zero_bias)
    if key not in _CACHE:
        nc = bacc.Bacc("TRN2", target_bir_lowering=False, debug=False,
                       num_devices=NCORES)
        build_core_program(nc, n, zero_bias=zero_bias)
        nc.compile()
        _CACHE[key] = nc
    return _CACHE[key]


_ONES = np.ones((1, P), ml_dtypes.bfloat16)


def _make_cmblk(scale_sq):
    # cmblk[p', pr, l] = s²_{2pr+a} iff l == 2pr+a and p' in head-a block
    cm = np.zeros((P, PAIRS, 8), np.float32)
    for pr in range(PAIRS):
        for a in range(2):
            cm[a * HD:(a + 1) * HD, pr, 2 * pr + a] = scale_sq[2 * pr + a]
    return cm.astype(ml_dtypes.bfloat16)


_IND8 = np.zeros((8, PAIRS, P), ml_dtypes.bfloat16)
for _pr in range(PAIRS):
    for _a in range(2):
        _IND8[2 * _pr + _a, _pr, _a * HD:(_a + 1) * HD] = 1.0


def make_in_maps(x, Wq, bq, Wk, bk, Wv, bv, Wo, bo, m):
    n = x.shape[1]
    sig = 1.0 / (1.0 + np.exp(-np.asarray(m, np.float64)))
    scale = np.float64(n) ** sig  # [16] per-head n^sigmoid(m)
    in_maps = []
    for c in range(NCORES):
        bi, g = divmod(c, 2)
        sl = slice(g * FG, (g + 1) * FG)
        hsc = scale[g * (H // G):(g + 1) * (H // G)]  # 8 local heads
        xa = np.asarray(x[bi], np.float32)
        in_maps.append({
            "xt": np.ascontiguousarray(
                xa.T.reshape(KT, P, n // 512, 512).transpose(1, 2, 0, 3)
                .astype(ml_dtypes.bfloat16)),
            "wq": np.ascontiguousarray(
                np.asarray(Wq, np.float32)[:, sl].reshape(KT, P, PAIRS, P)
                .transpose(1, 0, 2, 3).astype(ml_dtypes.bfloat16)),
            "wk": np.ascontiguousarray(
                np.asarray(Wk, np.float32)[:, sl].reshape(KT, P, FG)
                .transpose(1, 0, 2).astype(ml_dtypes.bfloat16)),
            "wv": np.ascontiguousarray(
                np.asarray(Wv, np.float32)[:, sl].reshape(KT, P, FG)
                .transpose(1, 0, 2).astype(ml_dtypes.bfloat16)),
            "wo": np.ascontiguousarray(
                np.asarray(Wo, np.float32)[sl].reshape(PAIRS, P, F)
                .transpose(1, 0, 2).astype(ml_dtypes.bfloat16)),
            "bq": np.ascontiguousarray(
                np.asarray(bq, np.float32)[sl].reshape(PAIRS, P).T),
            "bk": np.ascontiguousarray(np.asarray(bk, np.float32)[sl].astype(ml_dtypes.bfloat16))[None, :],
            "bv": np.ascontiguousarray(np.asarray(bv, np.float32)[sl].astype(ml_dtypes.bfloat16))[None, :],
            "cmblk": _make_cmblk((hsc ** 2).astype(np.float64)),
            "ind8": _IND8,
            "cones": _ONES,
        })
    return in_maps


def kernel(x, Wq, bq, Wk, bk, Wv, bv, Wo, bo, m, _trace=False):
    x = np.asarray(x, np.float32)
    b, n, f = x.shape
    zb = (not np.any(np.asarray(bq))) and (not np.any(np.asarray(bk))) \
        and (not np.any(np.asarray(bv)))
    nc = get_nc(n, zero_bias=zb)
    in_maps = make_in_maps(x, Wq, bq, Wk, bk, Wv, bv, Wo, bo, m)
    res = bass_utils.run_bass_kernel_spmd(nc, in_maps,
                                          core_ids=list(range(NCORES)),
                                          trace=_trace)
    outs = [r["out"].reshape(n, f) for r in res.results]
    y = np.empty((b, n, f), np.float32)
    for bi in range(b):
        y[bi] = outs[2 * bi].astype(np.float32) + \
            outs[2 * bi + 1].astype(np.float32)
    y += np.asarray(bo, np.float32).reshape(1, 1, f)
    if _trace:
        kernel._last_results = res
    return y


if __name__ == "__main__":
    # build-only smoke test (no device)
    for zb in (True, False):
        nc = bacc.Bacc("TRN2", target_bir_lowering=False, debug=False,
                       num_devices=NCORES)
        build_core_program(nc, n=2048, zero_bias=zb)
        nc.compile()
        print(f"build OK zero_bias={zb}")


# revision 68
# speedup vs baseline: 1.0036x; 1.0036x over previous
"""Multi-head cosine self-attention on 8 Trainium2 NeuronCores (Bass/Tile).

Problem: y = MHA(x) with L2-normalized q/k (cosine attention) and per-head
scaling sim / n**sigmoid(m);  x: [4, 2048, 1024], 16 heads of dim 64.

There is no softmax, so attention is LINEAR and can be reassociated:
    out_h = (q̂_h k̂_hᵀ / s_h) v_h = q̃_h (k_hᵀ ṽ_h)
with  ṽ_h = v_h / ||k_j||  (k-norms folded into v rows) and
      q̃_h = q_h / (s_h ||q_i||)  (q-norms and head scale folded into q).
The O(n²) sim/attn matrices disappear; per head only a 64×64 kv product
remains.  Folding further, per-head A_h = kv_hᵀ Wo_h stacks into M [512, F]
so the entire attention + output projection collapses to  y_part = q̃ M.

Sharding: core c handles batch c//2 and head-group c%2 (8 heads = 512 of
the 1024 q/k/v features).  Host sums the two partials per batch, adds bo.

Per-core pipeline (n=2048; matmul operands bf16, PSUM f32).  HW profile
shows the PE streams N=512 matmuls back-to-back at ~216 ns once warm, so
the design minimizes everything around the matmul stream:
  - the startup-critical k-burst inputs (wk + x chunk 0) are spread over
    ALL THREE DMA queues in 256KB slices (per-queue bandwidth saturates
    below the core's share); wv/wq queue strictly behind them, x
    prefetches ride behind wq, wo is deferred to chunk 1.  x and out use
    chunk/tile-major DRAM layouts so every DMA is contiguous.
  - within each projection tile pair the contraction loop runs k8-outer
    so matmul k8 consumes exactly the weight slice the DMA just
    delivered (chunk 0 tracks the arrival curve).
  - ONE unified PSUM pool for all phases (tags: big [P,2,512]x3 = 6
    banks, kv 1 bank, qn 1 bank) so there is no pool-transition barrier
    between the projection phase and the M/output phases.
  - q-norm scale = Abs_reciprocal_sqrt in ONE scalar-engine op (bf16
    out) — the old Sqrt + DVE-reciprocal chain cost 3.3us on the
    8-partition [8,512] layout and stalled the PE FIFO at the phase
    transition.  ind8/qnr are bf16 so the broadcast matmuls run at
    normal bf16 speed instead of fp32-HIGH.
  - transition order: kv block-diagonal evicts first (vector), last
    q-norm reduce, then M = blkT Wo matmuls cover the rsqrt latency,
    then the last q-scale apply, then phase 4; the last chunk's qT
    copies are deferred past the rsqrt in the scalar FIFO.
  - phase-4 output tiles evict per 512-feature half (DVE fc0 / ACT fc1)
    into one contiguous staging tile, then ONE 256KB DMA per n-tile, all
    on the sync queue (gpsimd then has no outstanding transfers at
    program end, collapsing its ~3us epilogue DRAIN); tile-major DRAM
    layout so the host reshape is a view, last tile in two halves.
"""

import os
import sys

for _p in ("/opt/trn_rl_repo",):
    if os.path.isdir(_p) and _p not in sys.path:
        sys.path.insert(0, _p)

from contextlib import ExitStack

import ml_dtypes
import numpy as np

import concourse.bacc as bacc
import concourse.mybir as mybir
import concourse.tile as tile
from concourse import bass_utils

P = 128
F = 1024  # model dim
H = 16  # total heads
HD = 64  # head dim
G = 2  # head groups (tensor-parallel factor)
FG = F // G  # 512 features per core
PAIRS = FG // P  # 4 head-pairs per core
KT = F // P  # 8 contraction tiles for the projections
NCORES = 8
F32 = mybir.dt.float32
FR = mybir.dt.float32r
BF = mybir.dt.bfloat16
AF = mybir.ActivationFunctionType
MUL = mybir.AluOpType.mult
ADD = mybir.AluOpType.add


def _mm(nc, out, lhsT, rhs, **kw):
    nc.tensor.matmul(out, lhsT, rhs, **kw)


def build_core_program(nc, n=2048, reps=1, zero_bias=True):
    NC = n // 512  # i-chunks
    NT = n // P  # n-tiles
    NTC = 512 // P  # n-tiles per i-chunk

    # chunk-major x layout: each 512-row chunk is a contiguous 8KB/partition
    # block, so chunk DMAs stream at full line efficiency
    xt = nc.dram_tensor("xt", [P, n // 512, KT, 512], BF,
                        kind="ExternalInput").ap()
    wq = nc.dram_tensor("wq", [P, KT, PAIRS, P], BF, kind="ExternalInput").ap()
    wk = nc.dram_tensor("wk", [P, KT, FG], BF, kind="ExternalInput").ap()
    wv = nc.dram_tensor("wv", [P, KT, FG], BF, kind="ExternalInput").ap()
    wo = nc.dram_tensor("wo", [P, PAIRS, F], BF, kind="ExternalInput").ap()
    bqd = nc.dram_tensor("bq", [P, PAIRS], F32, kind="ExternalInput").ap()
    bkd = nc.dram_tensor("bk", [1, FG], BF, kind="ExternalInput").ap()
    bvd = nc.dram_tensor("bv", [1, FG], BF, kind="ExternalInput").ap()
    # cmblk[:, p, l]: s²_{2p+a} on the partition block of head a=l-2p (else 0)
    cmblk = nc.dram_tensor("cmblk", [P, PAIRS, 8], BF, kind="ExternalInput").ap()
    # ind8[l, p, m] = 1 iff l == 2p + m//64  (partition-broadcast selector)
    ind8 = nc.dram_tensor("ind8", [8, PAIRS, P], BF, kind="ExternalInput").ap()
    cones = nc.dram_tensor("cones", [1, P], BF, kind="ExternalInput").ap()
    # tile-major out layout: each [128, 1024] n-tile is one contiguous
    # 256KB block (host reshape is a no-op view)
    out = nc.dram_tensor("out", [n // P, P, F], BF,
                         kind="ExternalOutput").ap()

    with tile.TileContext(nc) as tc, ExitStack() as ctx:
        const = ctx.enter_context(tc.tile_pool(name="const", bufs=1))
        persist = ctx.enter_context(tc.tile_pool(name="persist", bufs=1))
        work = ctx.enter_context(tc.tile_pool(name="work", bufs=1))

        cm_sb = const.tile([P, PAIRS, 8], BF)
        ind_sb = const.tile([8, PAIRS, P], BF)
        zcol = const.tile([P, 1], F32)
        nc.any.memset(zcol[:], 0.0)
        if not zero_bias:
            ones_row = const.tile([1, P], BF)
            nc.sync.dma_start(ones_row[:], cones)
            bq_sb = const.tile([P, PAIRS], F32)
            nc.sync.dma_start(bq_sb[:], bqd)
            bk_sb = const.tile([1, FG], BF)
            nc.sync.dma_start(bk_sb[:], bkd)
            bv_sb = const.tile([1, FG], BF)
            nc.sync.dma_start(bv_sb[:], bvd)

        # --- weights + first x chunk: per-k8 128KB slices so the first
        # matmuls can start as soon as slice 0 lands.  Queues: scalar =
        # wk then wq then cm/ind; sync = xc0 then later x prefetches +
        # half the out tiles; gpsimd = wv only; vector = wo (deferred to
        # chunk 1) + the other half of out tiles. ------------------------
        wk_sb = persist.tile([P, KT, FG], BF)
        wv_sb = persist.tile([P, KT, FG], BF)
        wq_sb = persist.tile([P, KT, PAIRS, P], BF)
        wo_sb = persist.tile([P, PAIRS, F], BF)
        xc0 = work.tile([P, KT, 512], BF, tag="x", bufs=3)
        # k-burst inputs (wk + xc0) spread over ALL THREE queues first —
        # per-queue bandwidth saturates well below the core's share, so
        # three concurrent streams feed the first matmuls fastest.  wv/wq
        # queue strictly behind them, cm/ind on the then-idle sync queue.
        nc.scalar.dma_start(wk_sb[:, 0:2], wk[:, 0:2])
        nc.sync.dma_start(xc0[:, 0:2], xt[:, 0, 0:2])
        nc.gpsimd.dma_start(wk_sb[:, 2:4], wk[:, 2:4])
        nc.scalar.dma_start(xc0[:, 2:4], xt[:, 0, 2:4])
        nc.sync.dma_start(wk_sb[:, 4:6], wk[:, 4:6])
        nc.gpsimd.dma_start(xc0[:, 4:6], xt[:, 0, 4:6])
        nc.scalar.dma_start(wk_sb[:, 6:8], wk[:, 6:8])
        nc.sync.dma_start(xc0[:, 6:8], xt[:, 0, 6:8])
        for k8 in range(0, KT, 2):
            nc.gpsimd.dma_start(wv_sb[:, k8:k8 + 2], wv[:, k8:k8 + 2])
        for k8 in range(0, KT, 2):
            nc.scalar.dma_start(wq_sb[:, k8:k8 + 2], wq[:, k8:k8 + 2])
        nc.sync.dma_start(cm_sb[:], cmblk)
        nc.sync.dma_start(ind_sb[:], ind8)

        # --- persistent activations -------------------------------------
        qT = persist.tile([P, PAIRS, n], BF)  # q̃^T (scaled in place)
        sq = persist.tile([P, PAIRS, n], BF)  # (q+bq)^2
        ksb = persist.tile([P, NT, FG], BF)  # k natural
        vsb = persist.tile([P, NT, FG], BF)  # ṽ natural (k-norms folded)
        knf = persist.tile([P, NT, 8], F32)  # 1/||k_j|| per local head
        blk = persist.tile([P, PAIRS, P], BF)  # block-diag kvT per pair
        nc.any.memset(blk[:], 0.0)
        msb = persist.tile([P, PAIRS, F], BF)  # M = kvT-blk^T @ Wo rows

        with tc.tile_pool(name="ps", bufs=1, space="PSUM") as ps:

            def qn_apply_block(ic, qnr):
                # broadcast across partitions + scale qT in place
                icsl = slice(ic * 512, (ic + 1) * 512)
                for h in range(PAIRS // 2):
                    bps = ps.tile([P, 2, 512], F32, tag="big", bufs=3)
                    for u in range(2):
                        pr = 2 * h + u
                        _mm(nc, bps[:, u, :], ind_sb[:, pr, :], qnr[:],
                            start=True, stop=True)
                    psl = slice(2 * h, 2 * h + 2)
                    nc.vector.tensor_tensor(qT[:, psl, icsl],
                                            qT[:, psl, icsl], bps, MUL)

            def qn_norm_block(ic):
                icsl = slice(ic * 512, (ic + 1) * 512)
                qnp = ps.tile([8, 512], F32, tag="qn", bufs=1)
                for pr in range(PAIRS):
                    _mm(nc, qnp, cm_sb[:, pr, :], sq[:, pr, icsl],
                        start=(pr == 0), stop=(pr == PAIRS - 1))
                qnr = work.tile([8, 512], BF, tag="qnr", bufs=2)
                nc.scalar.activation(qnr[:], qnp, AF.Abs_reciprocal_sqrt,
                                     bias=zcol[:8])
                return qnr

            for _rep in range(reps):  # reps>1: device-time measurement only
                # ========= phase 1: projections + norms, per i-chunk =====
                # The q-norm/scale block for chunk ic is deferred one chunk
                # so its PE matmuls never wait on the ACT/DVE chain.
                xc_next = xc0 if _rep == 0 else None
                for ic in range(NC):
                    icsl = slice(ic * 512, (ic + 1) * 512)
                    ntsl = slice(ic * NTC, (ic + 1) * NTC)
                    # x prefetches ride the scalar queue BEHIND wk/wq/cm:
                    # per-queue FIFO transfer order keeps them from stealing
                    # round-robin bandwidth from the startup-critical
                    # streams (sync=xc0, scalar=wk, gpsimd=wv).
                    if xc_next is None:
                        xc = work.tile([P, KT, 512], BF, tag="x", bufs=3)
                        nc.scalar.dma_start(xc[:], xt[:, ic])
                    else:
                        xc = xc_next
                    if ic + 1 < NC:  # prefetch next chunk
                        xc_next = work.tile([P, KT, 512], BF, tag="x", bufs=3)
                        nc.scalar.dma_start(xc_next[:], xt[:, ic + 1])
                    else:
                        xc_next = None
                    if ic == 1 and _rep == 0:
                        nc.gpsimd.dma_start(wo_sb[:], wo)

                    # ---- k tiles (natural layout), 2 per PSUM tile,
                    # k8-outer so chunk 0 tracks the wk DMA slices --------
                    for h in range(NTC // 2):
                        kp = ps.tile([P, 2, 512], F32, tag="big", bufs=3)
                        for k8 in range(KT):
                            for u in range(2):
                                jt = 2 * h + u
                                jsl = slice(jt * P, (jt + 1) * P)
                                _mm(nc, kp[:, u, :], xc[:, k8, jsl],
                                    wk_sb[:, k8, :], start=(k8 == 0),
                                    stop=(zero_bias and k8 == KT - 1))
                        if not zero_bias:
                            for u in range(2):
                                _mm(nc, kp[:, u, :], ones_row, bk_sb,
                                    start=False, stop=True)
                        nc.vector.tensor_copy(
                            ksb[:, ic * NTC + 2 * h:ic * NTC + 2 * h + 2, :], kp)
                    # deferred q-norm reduce of the previous chunk
                    if ic >= 1:
                        qnr_prev = qn_norm_block(ic - 1)
                    # ---- k-norm chain, whole chunk ----------------------
                    ksq = work.tile([P, NTC, FG], BF, tag="ksq", bufs=1)
                    nc.gpsimd.tensor_mul(ksq[:], ksb[:, ntsl, :],
                                         ksb[:, ntsl, :])
                    kn2 = work.tile([P, NTC * 8], F32, tag="kn2", bufs=1)
                    nc.vector.tensor_reduce(
                        kn2[:], ksq.rearrange("p t (h d) -> p t h d", h=8),
                        mybir.AxisListType.X, ADD)
                    nc.scalar.activation(
                        knf[:, ntsl, :].rearrange("p t h -> p (t h)"), kn2[:],
                        AF.Abs_reciprocal_sqrt, bias=zcol[:])
                    # ---- v tiles ----------------------------------------
                    for h in range(NTC // 2):
                        vp = ps.tile([P, 2, 512], F32, tag="big", bufs=3)
                        for k8 in range(KT):
                            for u in range(2):
                                jt = 2 * h + u
                                jsl = slice(jt * P, (jt + 1) * P)
                                _mm(nc, vp[:, u, :], xc[:, k8, jsl],
                                    wv_sb[:, k8, :], start=(k8 == 0),
                                    stop=(zero_bias and k8 == KT - 1))
                        if not zero_bias:
                            for u in range(2):
                                _mm(nc, vp[:, u, :], ones_row, bv_sb,
                                    start=False, stop=True)
                        nc.scalar.copy(
                            vsb[:, ic * NTC + 2 * h:ic * NTC + 2 * h + 2, :], vp)
                    # fold 1/||k_j|| into v rows (GpSimd, SBUF-only)
                    nc.gpsimd.tensor_tensor(
                        vsb[:, ntsl, :].rearrange("p t (h d) -> p t h d", h=8),
                        vsb[:, ntsl, :].rearrange("p t (h d) -> p t h d", h=8),
                        knf[:, ntsl, :].unsqueeze(-1).broadcast_to(
                            [P, NTC, 8, HD]), MUL)

                    # ---- qT pairs (transposed layout), 2 per PSUM tile --
                    # last chunk: Squares first, qT copies deferred past the
                    # transition rsqrt so it isn't stuck behind them in the
                    # scalar FIFO (the qp PSUM bufs are WAR-protected)
                    qp_last = []
                    for h in range(PAIRS // 2):
                        qp = ps.tile([P, 2, 512], F32, tag="big", bufs=3)
                        for k8 in range(KT):
                            for u in range(2):
                                pr = 2 * h + u
                                _mm(nc, qp[:, u, :], wq_sb[:, k8, pr, :],
                                    xc[:, k8, :], start=(k8 == 0),
                                    stop=(k8 == KT - 1))
                        psl = slice(2 * h, 2 * h + 2)
                        if zero_bias and ic == NC - 1:
                            # per-pair Squares: the qn-norm matmul for pair
                            # pr only needs sq[:, pr], so finer ACT ops let
                            # the qn accumulation start ~0.7us earlier than
                            # one [P,2,512] Square per h
                            for u in range(2):
                                pr = 2 * h + u
                                nc.scalar.activation(
                                    sq[:, pr:pr + 1, icsl],
                                    qp[:, u:u + 1, :], AF.Square,
                                    bias=zcol[:])
                            qp_last.append((qp, psl))
                        elif zero_bias:
                            # sq = qT*qT on the lightly-loaded GpSimd
                            # (SBUF-only) — halves the scalar engine's
                            # per-chunk load; the one-chunk-deferred qn
                            # block gives the longer chain plenty of slack
                            nc.scalar.copy(qT[:, psl, icsl], qp)
                            nc.gpsimd.tensor_mul(sq[:, psl, icsl],
                                                 qT[:, psl, icsl],
                                                 qT[:, psl, icsl])
                        else:
                            for u in range(2):
                                pr = 2 * h + u
                                nc.scalar.activation(
                                    qT[:, pr, icsl], qp[:, u, :], AF.Identity,
                                    bias=bq_sb[:, pr:pr + 1])
                                nc.scalar.activation(
                                    sq[:, pr, icsl], qp[:, u, :], AF.Square,
                                    bias=bq_sb[:, pr:pr + 1])

                    # ---- apply the previous chunk's q scaling -----------
                    if ic >= 1:
                        qn_apply_block(ic - 1, qnr_prev)

                    # ---- incremental kvT accumulation for this chunk ----
                    # kvT = ṽ^T k in one persistent PSUM bank: one full-pair
                    # matmul per (pair, n-tile) computes the two good
                    # diagonal 64×64 blocks (plus off-diagonal garbage that
                    # is never evicted), accumulated across all chunks.
                    if ic == 0:
                        kvp = ps.tile([P, PAIRS, P], F32, tag="kv", bufs=1)
                    for pr in range(PAIRS):
                        c0 = pr * P
                        for nt in range(ic * NTC, (ic + 1) * NTC):
                            # single start=True: the bank-wide has_written
                            # clear must precede ALL pairs' first writes
                            _mm(nc, kvp[:, pr, :], vsb[:, nt, c0:c0 + P],
                                ksb[:, nt, c0:c0 + P],
                                start=(nt == 0 and pr == 0),
                                stop=(nt == NT - 1),
                                skip_group_check=True)

                # ========= transition: last q-norm + kv evict + M ========
                # PE order: last chunk's qn reduce matmuls, then its apply
                # matmuls — DVE+ACT evict the kv diagonal blocks underneath
                # — then M = blkT Wo with no pool barrier.
                # both kv evicts on the (now light) scalar queue: vector's
                # chunk-3 backlog (k evicts + apply-TT + qT copy) was
                # delaying them ~1.3us, gating the M matmuls
                nc.scalar.copy(blk[0:HD, :, 0:HD], kvp[0:HD, :, 0:HD])
                nc.scalar.copy(blk[HD:P, :, HD:P], kvp[HD:P, :, HD:P])
                qnr_last = qn_norm_block(NC - 1)
                # both deferred qT copies on VECTOR: on scalar they queue
                # behind the rsqrt's wait and stall the M matmuls via the
                # PSUM-ring WAR (mp tiles alias the qp tiles)
                lsl = slice((NC - 1) * 512, NC * 512)
                for qp, psl in qp_last:
                    nc.vector.tensor_copy(qT[:, psl, lsl], qp)

                # ========= phase 3: M = blk^T @ Wo rows ==================
                # (emitted before the last qn apply: the PE FIFO then has
                # ~2us of M matmuls to chew on while the Rsqrt lands)
                for pr in range(PAIRS):
                    mp = ps.tile([P, 2, 512], F32, tag="big", bufs=3)
                    for fc in range(2):
                        fsl = slice(fc * 512, (fc + 1) * 512)
                        _mm(nc, mp[:, fc, :], blk[:, pr, :],
                            wo_sb[:, pr, fsl], start=True, stop=True)
                    # evict per 512-half, rotating over three engines, so
                    # phase 4's first accumulation group is unblocked as
                    # early as possible
                    for fc in range(2):
                        fsl = slice(fc * 512, (fc + 1) * 512)
                        if (2 * pr + fc) % 2 == 0:
                            nc.vector.tensor_copy(msb[:, pr, fsl], mp[:, fc, :])
                        else:
                            nc.scalar.copy(msb[:, pr, fsl], mp[:, fc, :])

                qn_apply_block(NC - 1, qnr_last)

                # ========= phase 4: y = q̃^T.T @ M, per n-tile ============
                for nt in range(NT):
                    ntsl = slice(nt * P, (nt + 1) * P)
                    op = ps.tile([P, 2, 512], F32, tag="big", bufs=3)
                    # one contiguous staging tile per n-tile: vector evicts
                    # the fc0 half, scalar the fc1 half, then a single DMA
                    # per tile (alternating queues; quarters on the last
                    # tile so the final drain is short)
                    ost = work.tile([P, 2, 512], BF, tag="ost", bufs=4)
                    for fc in range(2):
                        fsl = slice(fc * 512, (fc + 1) * 512)
                        for pr in range(PAIRS):
                            _mm(nc, op[:, fc, :], qT[:, pr, ntsl],
                                msb[:, pr, fsl],
                                start=(pr == 0), stop=(pr == PAIRS - 1))
                        if fc == 0:
                            nc.vector.tensor_copy(ost[:, 0, :], op[:, fc, :])
                        else:
                            nc.scalar.copy(ost[:, 1, :], op[:, fc, :])
                    # all out-DMAs on the sync queue: gpsimd then has no
                    # outstanding transfers at program end, so its epilogue
                    # DRAIN (measured ~3us) collapses
                    ostf = ost.rearrange("p u f -> p (u f)")
                    if nt == NT - 1:
                        nc.sync.dma_start(out[nt, :, 0:512], ost[:, 0, :])
                        nc.sync.dma_start(out[nt, :, 512:1024], ost[:, 1, :])
                    else:
                        nc.sync.dma_start(out[nt], ostf)
    return nc


_CACHE = {}


def get_nc(n=2048, zero_bias=True):
    key = (n, zero_bias)
    if key not in _CACHE:
        nc = bacc.Bacc("TRN2", target_bir_lowering=False, debug=False,
                       num_devices=NCORES)
        build_core_program(nc, n, zero_bias=zero_bias)
        nc.compile()
        _CACHE[key] = nc
    return _CACHE[key]


_ONES = np.ones((1, P), ml_dtypes.bfloat16)


def _make_cmblk(scale_sq):
    # cmblk[p', pr, l] = s²_{2pr+a} iff l == 2pr+a and p' in head-a block
    cm = np.zeros((P, PAIRS, 8), np.float32)
    for pr in range(PAIRS):
        for a in range(2):
            cm[a * HD:(a + 1) * HD, pr, 2 * pr + a] = scale_sq[2 * pr + a]
    return cm.astype(ml_dtypes.bfloat16)


_IND8 = np.zeros((8, PAIRS, P), ml_dtypes.bfloat16)
for _pr in range(PAIRS):
    for _a in range(2):
        _IND8[2 * _pr + _a, _pr, _a * HD:(_a + 1) * HD] = 1.0


def make_in_maps(x, Wq, bq, Wk, bk, Wv, bv, Wo, bo, m):
    n = x.shape[1]
    sig = 1.0 / (1.0 + np.exp(-np.asarray(m, np.float64)))
    scale = np.float64(n) ** sig  # [16] per-head n^sigmoid(m)
    in_maps = []
    for c in range(NCORES):
        bi, g = divmod(c, 2)
        sl = slice(g * FG, (g + 1) * FG)
        hsc = scale[g * (H // G):(g + 1) * (H // G)]  # 8 local heads
        xa = np.asarray(x[bi], np.float32)
        in_maps.append({
            "xt": np.ascontiguousarray(
                xa.T.reshape(KT, P, n // 512, 512).transpose(1, 2, 0, 3)
                .astype(ml_dtypes.bfloat16)),
            "wq": np.ascontiguousarray(
                np.asarray(Wq, np.float32)[:, sl].reshape(KT, P, PAIRS, P)
                .transpose(1, 0, 2, 3).astype(ml_dtypes.bfloat16)),
            "wk": np.ascontiguousarray(
                np.asarray(Wk, np.float32)[:, sl].reshape(KT, P, FG)
                .transpose(1, 0, 2).astype(ml_dtypes.bfloat16)),
            "wv": np.ascontiguousarray(
                np.asarray(Wv, np.float32)[:, sl].reshape(KT, P, FG)
                .transpose(1, 0, 2).astype(ml_dtypes.bfloat16)),
            "wo": np.ascontiguousarray(
                np.asarray(Wo, np.float32)[sl].reshape(PAIRS, P, F)
                .transpose(1, 0, 2).astype(ml_dtypes.bfloat16)),
            "bq": np.ascontiguousarray(
                np.asarray(bq, np.float32)[sl].reshape(PAIRS, P).T),
            "bk": np.ascontiguousarray(np.asarray(bk, np.float32)[sl].astype(ml_dtypes.bfloat16))[None, :],
            "bv": np.ascontiguousarray(np.asarray(bv, np.float32)[sl].astype(ml_dtypes.bfloat16))[None, :],
            "cmblk": _make_cmblk((hsc ** 2).astype(np.float64)),
            "ind8": _IND8,
            "cones": _ONES,
        })
    return in_maps


def kernel(x, Wq, bq, Wk, bk, Wv, bv, Wo, bo, m, _trace=False):
    x = np.asarray(x, np.float32)
    b, n, f = x.shape
    zb = (not np.any(np.asarray(bq))) and (not np.any(np.asarray(bk))) \
        and (not np.any(np.asarray(bv)))
    nc = get_nc(n, zero_bias=zb)
    in_maps = make_in_maps(x, Wq, bq, Wk, bk, Wv, bv, Wo, bo, m)
    res = bass_utils.run_bass_kernel_spmd(nc, in_maps,
                                          core_ids=list(range(NCORES)),
                                          trace=_trace)
    outs = [r["out"].reshape(n, f) for r in res.results]
    y = np.empty((b, n, f), np.float32)
    for bi in range(b):
        y[bi] = outs[2 * bi].astype(np.float32) + \
            outs[2 * bi + 1].astype(np.float32)
    y += np.asarray(bo, np.float32).reshape(1, 1, f)
    if _trace:
        kernel._last_results = res
    return y


if __name__ == "__main__":
    # build-only smoke test (no device)
    for zb in (True, False):
        nc = bacc.Bacc("TRN2", target_bir_lowering=False, debug=False,
                       num_devices=NCORES)
        build_core_program(nc, n=2048, zero_bias=zb)
        nc.compile()
        print(f"build OK zero_bias={zb}")
